# revision 2
# baseline (speedup 1.0000x reference)
"""Trainium2 Bass kernel v2 for nn_AMLDetector — instruction-count-minimal
redesign for the dispatch-bound axon execution path.

Key changes vs baseline:
  - SAGE aggregation: host lays out gathered neighbor features as
    xe [128, NPAD, L] (zero-padded per dst); device does ONE
    vector.tensor_reduce + recip scale (replaces dma_gather + 113 matmuls).
  - All 256-contraction matmuls use fp8e4m3 DoubleRow (2 k-tiles/instr).
    Weights are host-scaled by 16 for fp8 range; descaled in free slots of
    fused vector ops.
  - LayerNorm stats + softmax denominators via gpsimd.partition_all_reduce
    (output is broadcast to all partitions, so no separate broadcast step).
  - No scalar-engine activations except the 8 softmax Exps (+2 Sqrt per LN).
  - Softmax exp carries a x128 factor (folded into the mask bias) so the
    normalized attention matrix P*128 stays in fp8 normal range; the 1/128
    and 1/16-weight descales fold into the z1 residual fused op.
"""

import os
import sys

sys.path.insert(0, "/opt/trn_rl_repo")

import numpy as np
from ml_dtypes import bfloat16
try:
    from ml_dtypes import float8_e4m3 as f8e4
except ImportError:
    from ml_dtypes import float8_e4m3fn as f8e4

N, E, T = 6144, 98304, 32
D_IN, D, DFF, H, OUT = 128, 256, 256, 4, 2
DH = D // H
EPS = 1e-5
NCORES = 8
NSLOTS = T // NCORES
MASK_NEG = -60.0
LN128 = float(np.log(128.0))

_cache = {}


def _ceil(a, b):
    return -(-a // b)


def _bf(a):
    return np.ascontiguousarray(np.asarray(a, np.float32).astype(bfloat16))


def _f8(a):
    return np.ascontiguousarray(np.asarray(a, np.float32).astype(f8e4))


def _f32(a):
    return np.ascontiguousarray(np.asarray(a, np.float32))


def _percol(v):
    """[256] -> [128, 2] (chan c = kt*128 + p -> [p, kt])."""
    return _f32(np.asarray(v, np.float32).reshape(2, 128).T)


def _ktileT(w):
    """W [out, in(256)] -> lhsT/rhs DR layout [128(p_in), 2(kk), out]."""
    wt = np.asarray(w, np.float32).T          # [in, out]
    return wt.reshape(2, 128, wt.shape[1]).transpose(1, 0, 2)


def _prep(inputs):
    x = _f32(inputs["x"])
    ei = np.asarray(inputs["edge_index"]).astype(np.int64)
    ts = np.asarray(inputs["timesteps"]).astype(np.int64)

    order = np.argsort(ts, kind="stable")
    ts_sorted = ts[order]
    starts = np.searchsorted(ts_sorted, np.arange(T + 1))
    tcounts = np.diff(starts)

    rank = np.argsort(-tcounts, kind="stable")
    core_slots = [[] for _ in range(NCORES)]
    for j in range(NSLOTS):
        ids = rank[j * NCORES:(j + 1) * NCORES]
        seq = range(NCORES) if j % 2 == 0 else range(NCORES - 1, -1, -1)
        for c, tid in zip(seq, ids):
            core_slots[c].append(int(tid))
    for c in range(NCORES):
        core_slots[c].sort(key=lambda t: -tcounts[t])

    SLOT = max(256, 128 * _ceil(int(tcounts.max()) if len(tcounts) else 1, 128))
    assert SLOT == 256, f"SLOT={SLOT} unsupported by v2 layout"
    NPAD = NSLOTS * SLOT
    PT = NPAD // 128

    node_of = np.full((NCORES, NPAD), -1, np.int64)
    rsizes = np.zeros((NCORES, NSLOTS), np.int64)
    for c in range(NCORES):
        for j, tid in enumerate(core_slots[c]):
            nodes = order[starts[tid]:starts[tid + 1]]
            node_of[c, j * SLOT: j * SLOT + len(nodes)] = nodes
            rsizes[c, j] = len(nodes)
    pos_core = np.zeros(N, np.int64)
    pos_idx = np.zeros(N, np.int64)
    for c in range(NCORES):
        m = node_of[c] >= 0
        pos_core[node_of[c][m]] = c
        pos_idx[node_of[c][m]] = np.nonzero(m)[0]

    # ---- edges: per-core [128, NPAD, L] neighbor layout ----
    src, dst = ei[0], ei[1]
    deg = np.bincount(dst, minlength=N).astype(np.int64)
    L = int(deg.max())
    recip = (1.0 / np.maximum(deg, 1)).astype(np.float32)

    x_bf = x.astype(bfloat16)
    ecore = pos_core[dst]
    epos = pos_idx[dst]
    xe_list = []
    for c in range(NCORES):
        m = ecore == c
        es, ep = src[m], epos[m]
        o = np.argsort(ep, kind="stable")
        es, ep = es[o], ep[o]
        # j-index within each dst
        b = np.searchsorted(ep, np.arange(NPAD + 1))
        j = np.arange(len(ep)) - b[ep]
        xe_nm = np.zeros((NPAD, L, D_IN), bfloat16)
        xe_nm[ep, j] = x_bf[es]
        xe_list.append(np.ascontiguousarray(
            xe_nm.transpose(2, 0, 1)))          # [128, NPAD, L]

    xT = np.zeros((NCORES, 128, NPAD), np.float32)
    recipb = np.zeros((NCORES, 128, NPAD), np.float32)
    for c in range(NCORES):
        m = node_of[c] >= 0
        xT[c][:, m] = x[node_of[c][m]].T
        recipb[c][:, m] = recip[node_of[c][m]][None, :]
        recipb[c][:, ~m] = 1.0

    # mask bias: 8 * (ln128 + 0|-60) per (key-partition, b*2+mt)
    m128 = np.zeros((NCORES, 128, NPAD // 128), np.float32)
    for c in range(NCORES):
        for b in range(NSLOTS):
            r = rsizes[c, b]
            for mt in range(2):
                nreal = int(np.clip(r - mt * 128, 0, 128))
                col = 2 * b + mt
                m128[c, :nreal, col] = LN128
                m128[c, nreal:, col] = LN128 + MASK_NEG

    pmask = np.zeros((128, 2), np.float32)
    pmask[0:64, 0] = 1.0 / 16.0
    pmask[64:128, 1] = 1.0 / 16.0

    # ---- weights ----
    g = lambda k: _f32(inputs[k])
    wgt = {}
    wgt["wlT"] = _bf(g("lin_l_w").T)            # [128, 256]
    wgt["wrT"] = _bf(g("lin_r_w").T)
    wgt["bn_g"] = _percol(g("bn_gamma"))
    wgt["bn_b"] = _percol(g("bn_beta"))
    wgt["pmask"] = pmask
    flags = {}
    for l in range(2):
        ipw, ipb = g("in_proj_w")[l], g("in_proj_b")[l]
        Wq, Wk, Wv = ipw[:D], ipw[D:2 * D], ipw[2 * D:]
        bq, bk, bv = ipb[:D], ipb[D:2 * D], ipb[2 * D:]
        Wo, bo = g("out_w")[l], g("out_b")[l]
        bo_f = bo + Wo @ bv
        g1v, b1v = g("ln1_g")[l], g("ln1_b")[l]
        W1g = g("ff1_w")[l] * g1v[None, :]
        b1f = g("ff1_b")[l] + g("ff1_w")[l] @ b1v
        bz = g("ff2_b")[l] + b1v                 # ff2_b + beta1
        wgt[f"wqd{l}"] = _f8(_ktileT(16.0 * Wq))
        wgt[f"wkd{l}"] = _f8(_ktileT(16.0 * Wk))
        wgt[f"wvb{l}"] = _bf(_ktileT(Wv))
        wgt[f"wob{l}"] = _bf(_ktileT(Wo))
        wgt[f"w1b{l}"] = _bf(_ktileT(W1g))
        wgt[f"w2b{l}"] = _bf(_ktileT(g("ff2_w")[l]))
        wgt[f"bq16{l}"] = _percol(16.0 * bq)
        wgt[f"bk16{l}"] = _percol(16.0 * bk)
        wgt[f"bo{l}"] = _percol(bo_f)
        wgt[f"b1e{l}"] = _percol(b1f)
        wgt[f"bz{l}"] = _percol(bz)
        wgt[f"g1{l}"] = _percol(g1v)
        wgt[f"g2{l}"] = _percol(g("ln2_g")[l])
        wgt[f"bz2{l}"] = _percol(g("ln2_b")[l])
        flags[f"bo{l}"] = bool(np.abs(bo_f).max() > 0)
        flags[f"bz{l}"] = bool(np.abs(bz).max() > 0)
        flags[f"bz2{l}"] = bool(np.abs(g("ln2_b")[l]).max() > 0)
    clsg = g("cls_w") * g("ln2_g")[1][None, :]
    clsb2 = g("cls_b") + g("cls_w") @ g("ln2_b")[1]
    cd = np.zeros((D, 8), np.float32)
    cd[:, :OUT] = clsg.T
    wgt["clsd"] = _bf(cd.reshape(2, 128, 8).transpose(1, 0, 2))
    wgt["clsb2"] = _f32(clsb2.reshape(OUT, 1))

    meta = dict(SLOT=SLOT, NPAD=NPAD, PT=PT, L=L, flags=flags)

    in_maps = []
    for c in range(NCORES):
        mm = {
            "xe": np.ascontiguousarray(
                xe_list[c].reshape(128, NPAD * L)),
            "xT": _bf(xT[c]),
            "recipb": _f32(recipb[c]),
            "m128": _f32(m128[c]),
        }
        for k, v in wgt.items():
            mm[k] = v
        in_maps.append(mm)

    gather_info = dict(node_of=node_of)
    return meta, in_maps, gather_info


def _build(meta, repeats=1):
    import concourse.bass as _bass
    import concourse.bacc as bacc
    import concourse.bass_isa as bisa
    import concourse.tile as tile
    from concourse import mybir

    NPAD, PT, L = meta["NPAD"], meta["PT"], meta["L"]
    flags = meta["flags"]
    F32, BF16, I16 = mybir.dt.float32, mybir.dt.bfloat16, mybir.dt.int16
    FP8 = mybir.dt.float8e4
    AL = mybir.AluOpType
    AF = mybir.ActivationFunctionType
    PM = mybir.MatmulPerfMode
    RO = bisa.ReduceOp
    NB = NSLOTS  # 4 attention blocks

    DBG = os.environ.get("KERNEL_DEBUG") or None

    nc = bacc.Bacc("TRN2", target_bir_lowering=False, debug=False,
                   enable_asserts=False, num_devices=NCORES)

    def din(name, shape, dt):
        return nc.dram_tensor(name, list(shape), dt, kind="ExternalInput").ap()

    xe_d = din("xe", (128, NPAD * L), BF16)
    xT_d = din("xT", (128, NPAD), BF16)
    recipb_d = din("recipb", (128, NPAD), F32)
    m128_d = din("m128", (128, PT), F32)
    wlT_d = din("wlT", (128, D), BF16)
    wrT_d = din("wrT", (128, D), BF16)
    WD = {}
    for nm in ("bn_g", "bn_b", "pmask"):
        WD[nm] = din(nm, (128, 2), F32)
    for l in range(2):
        for nm in ("wqd", "wkd"):
            WD[f"{nm}{l}"] = din(f"{nm}{l}", (128, 2, D), FP8)
        for nm in ("wvb", "wob", "w1b", "w2b"):
            WD[f"{nm}{l}"] = din(f"{nm}{l}", (128, 2, D), BF16)
        for nm in ("bq16", "bk16", "bo", "b1e", "bz", "g1", "g2", "bz2"):
            WD[f"{nm}{l}"] = din(f"{nm}{l}", (128, 2), F32)
    WD["clsd"] = din("clsd", (128, 2, 8), BF16)
    WD["clsb2"] = din("clsb2", (OUT, 1), F32)

    out_d = nc.dram_tensor("out", [OUT, NPAD], F32, kind="ExternalOutput").ap()
    if DBG:
        dbg_d = nc.dram_tensor("dbg", [128, 2 * NPAD], BF16,
                               kind="ExternalOutput").ap()

    def bc(sl, insert, tail=None):
        """Broadcast AP: insert [0, n] dims after partition dim of 2D slice."""
        ap = [list(sl.ap[0])]
        for n in insert:
            ap.append([0, n])
        for d in (tail if tail is not None else sl.ap[1:]):
            ap.append(list(d))
        return _bass.AP(tensor=sl.tensor, offset=sl.offset, ap=ap)

    with tile.TileContext(nc) as tc:
        import contextlib
        ctx = contextlib.ExitStack()
        consts = ctx.enter_context(tc.tile_pool(name="consts", bufs=1))
        work = ctx.enter_context(tc.tile_pool(name="work", bufs=1))
        psA = ctx.enter_context(tc.tile_pool(name="psA", bufs=1, space="PSUM"))
        psB = ctx.enter_context(tc.tile_pool(name="psB", bufs=2, space="PSUM"))
        dram = ctx.enter_context(tc.tile_pool(name="dram", bufs=1, space="DRAM"))

        def load(name, ap, shape, dt):
            t = consts.tile(list(shape), dt, tag=name)
            nc.sync.dma_start(t[:], ap[:])
            return t

        xe = load("xe", xe_d, (128, NPAD, L), BF16)
        xT = load("xT", xT_d, (128, NPAD), BF16)
        recipb = load("recipb", recipb_d, (128, NPAD), F32)
        m128 = load("m128", m128_d, (128, PT), F32)
        wlT = load("wlT", wlT_d, (128, D), BF16)
        wrT = load("wrT", wrT_d, (128, D), BF16)
        W = {}
        for nm in ("bn_g", "bn_b", "pmask"):
            W[nm] = load(nm, WD[nm], (128, 2), F32)
        for l in range(2):
            for nm in ("wqd", "wkd"):
                W[f"{nm}{l}"] = load(f"{nm}{l}", WD[f"{nm}{l}"],
                                     (128, 2, D), FP8)
            for nm in ("wvb", "wob", "w1b", "w2b"):
                W[f"{nm}{l}"] = load(f"{nm}{l}", WD[f"{nm}{l}"],
                                     (128, 2, D), BF16)
            for nm in ("bq16", "bk16", "bo", "b1e", "bz", "g1", "g2", "bz2"):
                W[f"{nm}{l}"] = load(f"{nm}{l}", WD[f"{nm}{l}"], (128, 2), F32)
        W["clsd"] = load("clsd", WD["clsd"], (128, 2, 8), BF16)
        W["clsb2"] = load("clsb2", WD["clsb2"], (OUT, 1), F32)

        TEN, VEC, SCA, GPS = nc.tensor, nc.vector, nc.scalar, nc.gpsimd

        epsc = consts.tile([128, 1], F32, tag="epsc", name="epsc")
        VEC.memset(epsc[:], EPS)
        eps2c = consts.tile([128, 1], F32, tag="eps2c", name="eps2c")
        VEC.memset(eps2c[:], float(D * D * EPS))

        qmb = work.tile([128, 2, 2, 2, NPAD], FP8, tag="qmb", name="qmb")
        VEC.memset(qmb[:].rearrange("p a b c d -> p (a b c d)"), 0.0)
        vz = work.tile([128, PT, 2, 2, 128], BF16, tag="vz", name="vz")
        VEC.memset(vz[:].rearrange("p a b c d -> p (a b c d)"), 0.0)

        if DBG:
            dbg_done = [False]

            def dbg_dump(name, ap2d):
                if DBG != name or dbg_done[0]:
                    return
                dbg_done[0] = True
                p = ap2d.partition_size()
                f = min(ap2d.free_size(), 2 * NPAD)
                GPS.dma_start(dbg_d[0:p, 0:f], ap2d)
        else:
            def dbg_dump(name, ap2d):
                return

        def ln_block(zw, tag):
            """zw [128, 2, NPAD] bf16 -> (t [128,2,NPAD] bf16, rstdr [128,NPAD] f32)
            with z_norm = t * rstdr."""
            S1 = work.tile([128, NPAD], F32, tag="ln_S1", name="ln_S1")
            mv = work.tile([128, NPAD], F32, tag="ln_mv", name="ln_mv")
            GPS.partition_all_reduce(S1[:], zw[:, 0, :].opt(), 128, RO.add)
            GPS.partition_all_reduce(mv[:], zw[:, 1, :].opt(), 128, RO.add)
            VEC.tensor_tensor(S1[:], S1[:], mv[:], AL.add)
            sqd = work.tile([128, 2, NPAD], BF16, tag="ln_sqd", name="ln_sqd")
            VEC.tensor_tensor(sqd[:], zw[:], zw[:], AL.mult)
            S2 = work.tile([128, NPAD], F32, tag="ln_S2", name="ln_S2")
            GPS.partition_all_reduce(S2[:], sqd[:, 0, :].opt(), 128, RO.add)
            GPS.partition_all_reduce(mv[:], sqd[:, 1, :].opt(), 128, RO.add)
            VEC.tensor_tensor(S2[:], S2[:], mv[:], AL.add)
            VEC.tensor_tensor(mv[:], S1[:], S1[:], AL.mult)
            VEC.scalar_tensor_tensor(mv[:], S2[:], float(D), mv[:],
                                     AL.mult, AL.subtract)
            srt = work.tile([128, NPAD], F32, tag="ln_srt", name="ln_srt")
            SCA.activation(srt[:], mv[:], AF.Sqrt,
                           bias=eps2c[:, 0:1], scale=1.0)
            rstdr = work.tile([128, NPAD], F32, tag=f"rstdr{tag}",
                              name=f"rstdr{tag}")
            VEC.reciprocal(rstdr[:], srt[:])
            t = work.tile([128, 2, NPAD], BF16, tag=f"lnt{tag}",
                          name=f"lnt{tag}")
            VEC.scalar_tensor_tensor(t[:], zw[:], float(D),
                                     bc(S1[:], [2]), AL.mult, AL.subtract)
            return t, rstdr

        for rep in range(repeats):
            # ============ SAGE ============
            aggT = work.tile([128, NPAD], F32, tag="ln_S2", name="aggT")
            VEC.tensor_reduce(aggT[:], xe[:], mybir.AxisListType.X, AL.add)
            aggTb = work.tile([128, NPAD], BF16, tag="uw", name="aggTb")
            VEC.tensor_tensor(aggTb[:], aggT[:], recipb[:], AL.mult)
            dbg_dump("agg", aggTb[:])

            # ============ h + BN ============
            h_ps = psA.tile([128, 2, NPAD], F32, tag="psA", name="psA")
            for kt in range(2):
                for ch in range(2):
                    sl = slice(ch * 512, (ch + 1) * 512)
                    TEN.matmul(h_ps[:, kt, sl],
                               wlT[:, kt * 128:(kt + 1) * 128],
                               aggTb[:, sl], start=True, stop=False)
                    TEN.matmul(h_ps[:, kt, sl],
                               wrT[:, kt * 128:(kt + 1) * 128],
                               xT[:, sl], start=False, stop=True)
            hT = work.tile([128, 2, NPAD], BF16, tag="lnt2", name="hT")
            VEC.tensor_copy(hT[:].rearrange("p a b -> p (a b)"),
                            h_ps[:].rearrange("p a b -> p (a b)"))
            ssum = work.tile([128, 4], F32, tag="ssum", name="ssum")
            VEC.tensor_reduce(ssum[:, 0:2], hT[:], mybir.AxisListType.X,
                              AL.add)
            sqd = work.tile([128, 2, NPAD], BF16, tag="ln_sqd", name="bn_sqd")
            VEC.tensor_tensor(sqd[:], hT[:], hT[:], AL.mult)
            VEC.tensor_reduce(ssum[:, 2:4], sqd[:], mybir.AxisListType.X,
                              AL.add)

            bn_in = dram.tile([128, 4], F32, tag="bn_in", name="bn_in")
            bn_out = dram.tile([128, 4], F32, tag="bn_out", name="bn_out")
            GPS.dma_start(bn_in[:], ssum[:])
            GPS.collective_compute(
                "AllReduce", AL.add,
                replica_groups=[list(range(NCORES))],
                ins=[bn_in[:].opt()], outs=[bn_out[:].opt()])
            ssg = work.tile([128, 4], F32, tag="ssg", name="ssg")
            GPS.dma_start(ssg[:], bn_out[:])

            cf = work.tile([128, 8], F32, tag="bncf", name="bncf")
            mean, e2 = cf[:, 0:2], cf[:, 2:4]
            a_bn, b_bn = cf[:, 4:6], cf[:, 6:8]
            VEC.tensor_scalar_mul(mean, ssg[:, 0:2], 1.0 / N)
            VEC.tensor_scalar_mul(e2, ssg[:, 2:4], 1.0 / N)
            var = work.tile([128, 2], F32, tag="bnvar", name="bnvar")
            VEC.tensor_tensor(var[:], mean, mean, AL.mult)
            VEC.tensor_tensor(var[:], e2, var[:], AL.subtract)
            SCA.activation(var[:], var[:], AF.Sqrt, bias=epsc[:, 0:1], scale=1.0)
            rbn = work.tile([128, 2], F32, tag="bnr", name="bnr")
            VEC.reciprocal(rbn[:], var[:])
            VEC.tensor_tensor(a_bn, W["bn_g"][:], rbn[:], AL.mult)
            VEC.tensor_tensor(b_bn, mean, a_bn, AL.mult)
            VEC.tensor_tensor(b_bn, W["bn_b"][:], b_bn, AL.subtract)

            h1 = work.tile([128, 2, NPAD], BF16, tag="inT", name="h1")
            for kt in range(2):
                VEC.tensor_scalar(h1[:, kt, :].opt(), hT[:, kt, :].opt(),
                                  a_bn[:, kt:kt + 1], b_bn[:, kt:kt + 1],
                                  AL.mult, AL.add)
            VEC.tensor_scalar_max(
                h1[:].rearrange("p a b -> p (a b)"),
                h1[:].rearrange("p a b -> p (a b)"), 0.0)
            h1f = work.tile([128, 2, NPAD], FP8, tag="inF", name="h1f")
            VEC.tensor_copy(h1f[:].rearrange("p a b -> p (a b)"),
                            h1[:].rearrange("p a b -> p (a b)"))
            dbg_dump("h1", h1[:].rearrange("p a b -> p (a b)"))

            inT, inF = h1, h1f
            # ============ layers ============
            for l in range(2):
                # ---- q ----
                q_ps = psA.tile([128, 2, NPAD], F32, tag="psA", name="psA")
                for kt in range(2):
                    for ch in range(2):
                        sl = slice(ch * 512, (ch + 1) * 512)
                        TEN.matmul(q_ps[:, kt, sl],
                                   W[f"wqd{l}"][:, :, kt * 128:(kt + 1) * 128],
                                   inF[:, :, sl], start=True, stop=True,
                                   perf_mode=PM.DoubleRow)
                dbg_dump("q", q_ps[:].rearrange("p a b -> p (a b)"))
                for hp in range(2):
                    for par in range(2):
                        VEC.scalar_tensor_tensor(
                            qmb[:, hp, par, hp, :].opt(),
                            q_ps[:, hp, :].opt(),
                            W[f"bq16{l}"][:, hp:hp + 1],
                            bc(W["pmask"][:, par:par + 1], [], [[0, NPAD]]),
                            AL.add, AL.mult)
                # ---- k ----
                k_ps = psA.tile([128, 2, NPAD], F32, tag="psA", name="psA")
                for kt in range(2):
                    for ch in range(2):
                        sl = slice(ch * 512, (ch + 1) * 512)
                        TEN.matmul(k_ps[:, kt, sl],
                                   W[f"wkd{l}"][:, :, kt * 128:(kt + 1) * 128],
                                   inF[:, :, sl], start=True, stop=True,
                                   perf_mode=PM.DoubleRow)
                kT = work.tile([128, 2, NPAD], FP8, tag="kT", name="kT")
                for kt in range(2):
                    VEC.tensor_scalar(kT[:, kt, :].opt(), k_ps[:, kt, :].opt(),
                                      W[f"bk16{l}"][:, kt:kt + 1], 1.0 / 16.0,
                                      AL.add, AL.mult)
                dbg_dump("kT", kT[:].rearrange("p a b -> p (a b)"))
                # ---- V (plain bf16, zero-padded per-(kt,par) blocks) ----
                for pt in range(PT):
                    v_ps = psB.tile([128, D], F32, tag="sm", name="psV")
                    for kk in range(2):
                        TEN.matmul(v_ps[:],
                                   inT[:, kk, pt * 128:(pt + 1) * 128].opt(),
                                   W[f"wvb{l}"][:, kk, :],
                                   start=(kk == 0), stop=(kk == 1))
                    vv = v_ps[:].rearrange("p (a b c) -> p a b c", a=2, b=2)
                    for par in range(2):
                        VEC.tensor_scalar_mul(
                            vz[:, pt, :, par, 64 * par:64 * par + 64].opt(),
                            vv[:, :, par, :].opt(), 1.0)
                # ---- scores + exp ----
                expw = work.tile([128, NB, 2, 2, 2, 256], BF16, tag="expw",
                                 name="expw")
                for b in range(NB):
                    for mt in range(2):
                        sc_ps = psB.tile([128, 2, 2, 256], F32, tag="sm",
                                         name="psS")
                        kc = (2 * b + mt) * 128
                        for par in range(2):
                            TEN.matmul(sc_ps[:, par, :, :],
                                       kT[:, :, kc:kc + 128],
                                       qmb[:, :, par, :,
                                           b * 256:(b + 1) * 256],
                                       start=True, stop=True,
                                       perf_mode=PM.DoubleRow)
                        col = 2 * b + mt
                        SCA.activation(
                            expw[:, b, mt, :, :, :].rearrange(
                                "p a b c -> p (a b c)"),
                            sc_ps[:].rearrange("p a b c -> p (a b c)"),
                            AF.Exp, bias=m128[:, col:col + 1], scale=0.125)
                dbg_dump("exp", expw[:, 0, :, :, :, :].rearrange("p a b c d -> p (a b c d)"))
                # ---- denom -> P~ (x128, fp8) ----
                den = work.tile([128, NB, 2, 2, 2, 256], BF16, tag="den",
                                name="den")
                GPS.partition_all_reduce(
                    den[:].rearrange("p a b c d e -> p (a b c d e)"),
                    expw[:].rearrange("p a b c d e -> p (a b c d e)"),
                    128, RO.add)
                den2 = work.tile([128, NB, 2, 2, 256], F32, tag="den2",
                                 name="den2")
                VEC.tensor_tensor(den2[:].opt(), den[:, :, 0, :, :, :].opt(),
                                  den[:, :, 1, :, :, :].opt(), AL.add)
                VEC.reciprocal(den2[:].rearrange("p a b c d -> p (a b c d)"),
                               den2[:].rearrange("p a b c d -> p (a b c d)"))
                for b in range(NB):
                    d2b = den2[:, b, :, :, :].opt()
                    VEC.scalar_tensor_tensor(
                        expw[:, b, :, :, :, :].opt(),
                        expw[:, b, :, :, :, :].opt(), 128.0,
                        bc(d2b, [2]), AL.mult, AL.mult)
                dbg_dump("ptil", expw[:, 0, :, :, :, :].rearrange("p a b c d -> p (a b c d)"))
                # ---- ovT ----
                voT = work.tile([128, 2, NPAD], BF16, tag="ln_sqd",
                                 name="voT")
                for b in range(NB):
                    ops = psB.tile([128, 2, 256], F32, tag="sm", name="psO")
                    for kt in range(2):
                        for par in range(2):
                            for mt in range(2):
                                TEN.matmul(
                                    ops[:, kt, :],
                                    vz[:, 2 * b + mt, kt, par, :].opt(),
                                    expw[:, b, mt, par, kt, :].opt(),
                                    start=(par == 0 and mt == 0),
                                    stop=(par == 1 and mt == 1))
                    VEC.tensor_copy(voT[:, :, b * 256:(b + 1) * 256], ops[:])
                dbg_dump("voT", voT[:].rearrange("p a b -> p (a b)"))
                # ---- oproj + residual -> z1 ----
                o_ps = psA.tile([128, 2, NPAD], F32, tag="psA", name="psA")
                for kt in range(2):
                    for ch in range(2):
                        sl = slice(ch * 512, (ch + 1) * 512)
                        for kk in range(2):
                            TEN.matmul(
                                o_ps[:, kt, sl],
                                W[f"wob{l}"][:, kk,
                                             kt * 128:(kt + 1) * 128],
                                voT[:, kk, sl].opt(),
                                start=(kk == 0), stop=(kk == 1))
                z1 = work.tile([128, 2, NPAD], BF16, tag="z1", name="z1")
                for kt in range(2):
                    VEC.scalar_tensor_tensor(z1[:, kt, :].opt(),
                                             o_ps[:, kt, :].opt(),
                                             1.0 / 128.0,
                                             inT[:, kt, :].opt(),
                                             AL.mult, AL.add)
                if flags[f"bo{l}"]:
                    for kt in range(2):
                        VEC.tensor_scalar(z1[:, kt, :].opt(), z1[:, kt, :].opt(),
                                          W[f"bo{l}"][:, kt:kt + 1], None,
                                          AL.add)
                dbg_dump("z1", z1[:].rearrange("p a b -> p (a b)"))
                # ---- LN1 -> z1n ----
                t1, rstdr1 = ln_block(z1, "1")
                z1n = work.tile([128, 2, NPAD], BF16, tag="z1n", name="z1n")
                VEC.tensor_tensor(z1n[:], t1[:], bc(rstdr1[:], [2]), AL.mult)
                dbg_dump("z1n", z1n[:].rearrange("p a b -> p (a b)"))
                # ---- FFN1 ----
                f_ps = psA.tile([128, 2, NPAD], F32, tag="psA", name="psA")
                for kt in range(2):
                    for ch in range(2):
                        sl = slice(ch * 512, (ch + 1) * 512)
                        for kk in range(2):
                            TEN.matmul(
                                f_ps[:, kt, sl],
                                W[f"w1b{l}"][:, kk,
                                             kt * 128:(kt + 1) * 128],
                                z1n[:, kk, sl].opt(),
                                start=(kk == 0), stop=(kk == 1))
                relu1 = work.tile([128, 2, NPAD], BF16, tag="z1n",
                                  name="relu1")
                for kt in range(2):
                    VEC.tensor_scalar(relu1[:, kt, :].opt(),
                                      f_ps[:, kt, :].opt(),
                                      W[f"b1e{l}"][:, kt:kt + 1], 0.0,
                                      AL.add, AL.max)
                # ---- FFN2 + z2 ----
                f2_ps = psA.tile([128, 2, NPAD], F32, tag="psA", name="psA")
                for kt in range(2):
                    for ch in range(2):
                        sl = slice(ch * 512, (ch + 1) * 512)
                        for kk in range(2):
                            TEN.matmul(
                                f2_ps[:, kt, sl],
                                W[f"w2b{l}"][:, kk,
                                             kt * 128:(kt + 1) * 128],
                                relu1[:, kk, sl].opt(),
                                start=(kk == 0), stop=(kk == 1))
                z2 = work.tile([128, 2, NPAD], BF16, tag="z1", name="z2")
                uw = work.tile([128, NPAD], BF16, tag="uw", name="uw")
                for kt in range(2):
                    VEC.scalar_tensor_tensor(uw[:], t1[:, kt, :].opt(),
                                             W[f"g1{l}"][:, kt:kt + 1],
                                             rstdr1[:], AL.mult, AL.mult)
                    VEC.scalar_tensor_tensor(z2[:, kt, :].opt(),
                                             f2_ps[:, kt, :].opt(),
                                             1.0, uw[:],
                                             AL.mult, AL.add)
                if flags[f"bz{l}"]:
                    for kt in range(2):
                        VEC.tensor_scalar(z2[:, kt, :].opt(), z2[:, kt, :].opt(),
                                          W[f"bz{l}"][:, kt:kt + 1], None,
                                          AL.add)
                dbg_dump("z2", z2[:].rearrange("p a b -> p (a b)"))
                # ---- LN2 ----
                t2, rstdr2 = ln_block(z2, "2")
                if l == 0:
                    h2 = work.tile([128, 2, NPAD], BF16, tag="inT",
                                   name="h2")
                    for kt in range(2):
                        VEC.scalar_tensor_tensor(h2[:, kt, :].opt(),
                                                 t2[:, kt, :].opt(),
                                                 W["g20"][:, kt:kt + 1],
                                                 rstdr2[:], AL.mult, AL.mult)
                    if flags["bz20"]:
                        for kt in range(2):
                            VEC.tensor_scalar(h2[:, kt, :].opt(), h2[:, kt, :].opt(),
                                              W["bz20"][:, kt:kt + 1], None,
                                              AL.add)
                    h2f = work.tile([128, 2, NPAD], FP8, tag="inF",
                                    name="h2f")
                    VEC.tensor_copy(h2f[:].rearrange("p a b -> p (a b)"),
                                    h2[:].rearrange("p a b -> p (a b)"))
                    dbg_dump("h2", h2[:].rearrange("p a b -> p (a b)"))
                    inT, inF = h2, h2f
                else:
                    z2n = work.tile([128, 2, NPAD], BF16, tag="z1n",
                                    name="z2n")
                    VEC.tensor_tensor(z2n[:], t2[:], bc(rstdr2[:], [2]),
                                      AL.mult)
                    c_ps = psB.tile([8, NPAD], F32, tag="sm", name="psC")
                    for ch in range(2):
                        sl = slice(ch * 512, (ch + 1) * 512)
                        for kk in range(2):
                            TEN.matmul(c_ps[:, sl], W["clsd"][:, kk, :],
                                       z2n[:, kk, sl].opt(),
                                       start=(kk == 0), stop=(kk == 1))
                    outb = work.tile([OUT, NPAD], F32, tag="ln_S1",
                                     name="outb")
                    VEC.tensor_scalar(outb[:], c_ps[0:OUT, :], 1.0,
                                      W["clsb2"][:, 0:1], AL.mult, AL.add)
                    nc.sync.dma_start(out_d[:], outb[:])
        ctx.close()

    nc.finalize()
    return nc


def _run(meta, in_maps, repeats=1):
    from concourse import bass_utils
    key = (meta["NPAD"], meta["L"], repeats,
           os.environ.get("KERNEL_DEBUG") or None)
    if key not in _cache:
        _cache[key] = _build(meta, repeats=repeats)
    nc = _cache[key]
    res = bass_utils.run_bass_kernel_spmd(
        nc, in_maps, core_ids=list(range(NCORES)), trace=False)
    return res


def kernel(**inputs):
    meta, in_maps, gi = _prep(inputs)
    repeats = int(os.environ.get("KERNEL_REPEATS", "1"))
    res = _run(meta, in_maps, repeats=repeats)
    node_of = gi["node_of"]
    out = np.zeros((N, OUT), np.float32)
    for c in range(NCORES):
        m = node_of[c] >= 0
        out[node_of[c][m]] = np.ascontiguousarray(res.results[c]["out"].T)[m]
    return out


# revision 4
# speedup vs baseline: 1.3822x; 1.3822x over previous
"""Trainium2 Bass kernel v2 for nn_AMLDetector — instruction-count-minimal
redesign for the dispatch-bound axon execution path.

Key changes vs baseline:
  - SAGE aggregation: host lays out gathered neighbor features as
    xe [128, NPAD, L] (zero-padded per dst); device does ONE
    vector.tensor_reduce + recip scale (replaces dma_gather + 113 matmuls).
  - All 256-contraction matmuls use fp8e4m3 DoubleRow (2 k-tiles/instr).
    Weights are host-scaled by 16 for fp8 range; descaled in free slots of
    fused vector ops.
  - LayerNorm stats + softmax denominators via gpsimd.partition_all_reduce
    (output is broadcast to all partitions, so no separate broadcast step).
  - No scalar-engine activations except the 8 softmax Exps (+2 Sqrt per LN).
  - Softmax exp carries a x128 factor (folded into the mask bias) so the
    normalized attention matrix P*128 stays in fp8 normal range; the 1/128
    and 1/16-weight descales fold into the z1 residual fused op.
"""

import os
import sys

sys.path.insert(0, "/opt/trn_rl_repo")

import numpy as np
from ml_dtypes import bfloat16
try:
    from ml_dtypes import float8_e4m3 as f8e4
except ImportError:
    from ml_dtypes import float8_e4m3fn as f8e4

N, E, T = 6144, 98304, 32
D_IN, D, DFF, H, OUT = 128, 256, 256, 4, 2
DH = D // H
EPS = 1e-5
NCORES = 8
NSLOTS = T // NCORES
MASK_NEG = -60.0
LN128 = float(np.log(128.0))

_cache = {}


def _ceil(a, b):
    return -(-a // b)


def _bf(a):
    return np.ascontiguousarray(np.asarray(a, np.float32).astype(bfloat16))


def _f8(a):
    return np.ascontiguousarray(np.asarray(a, np.float32).astype(f8e4))


def _f32(a):
    return np.ascontiguousarray(np.asarray(a, np.float32))


def _percol(v):
    """[256] -> [128, 2] (chan c = kt*128 + p -> [p, kt])."""
    return _f32(np.asarray(v, np.float32).reshape(2, 128).T)


def _ktileT(w):
    """W [out, in(256)] -> lhsT/rhs DR layout [128(p_in), 2(kk), out]."""
    wt = np.asarray(w, np.float32).T          # [in, out]
    return wt.reshape(2, 128, wt.shape[1]).transpose(1, 0, 2)


def _prep(inputs):
    x = _f32(inputs["x"])
    ei = np.asarray(inputs["edge_index"]).astype(np.int64)
    ts = np.asarray(inputs["timesteps"]).astype(np.int64)

    order = np.argsort(ts, kind="stable")
    ts_sorted = ts[order]
    starts = np.searchsorted(ts_sorted, np.arange(T + 1))
    tcounts = np.diff(starts)

    rank = np.argsort(-tcounts, kind="stable")
    core_slots = [[] for _ in range(NCORES)]
    for j in range(NSLOTS):
        ids = rank[j * NCORES:(j + 1) * NCORES]
        seq = range(NCORES) if j % 2 == 0 else range(NCORES - 1, -1, -1)
        for c, tid in zip(seq, ids):
            core_slots[c].append(int(tid))
    for c in range(NCORES):
        core_slots[c].sort(key=lambda t: -tcounts[t])

    SLOT = max(256, 128 * _ceil(int(tcounts.max()) if len(tcounts) else 1, 128))
    assert SLOT == 256, f"SLOT={SLOT} unsupported by v2 layout"
    NPAD = NSLOTS * SLOT
    PT = NPAD // 128

    node_of = np.full((NCORES, NPAD), -1, np.int64)
    rsizes = np.zeros((NCORES, NSLOTS), np.int64)
    for c in range(NCORES):
        for j, tid in enumerate(core_slots[c]):
            nodes = order[starts[tid]:starts[tid + 1]]
            node_of[c, j * SLOT: j * SLOT + len(nodes)] = nodes
            rsizes[c, j] = len(nodes)
    pos_core = np.zeros(N, np.int64)
    pos_idx = np.zeros(N, np.int64)
    for c in range(NCORES):
        m = node_of[c] >= 0
        pos_core[node_of[c][m]] = c
        pos_idx[node_of[c][m]] = np.nonzero(m)[0]

    # ---- edges: per-core [128, NPAD, L] neighbor layout ----
    src, dst = ei[0], ei[1]
    deg = np.bincount(dst, minlength=N).astype(np.int64)
    L = int(deg.max())
    recip = (1.0 / np.maximum(deg, 1)).astype(np.float32)

    x_bf = x.astype(bfloat16)
    ecore = pos_core[dst]
    epos = pos_idx[dst]
    xe_list = []
    for c in range(NCORES):
        m = ecore == c
        es, ep = src[m], epos[m]
        o = np.argsort(ep, kind="stable")
        es, ep = es[o], ep[o]
        # j-index within each dst
        b = np.searchsorted(ep, np.arange(NPAD + 1))
        j = np.arange(len(ep)) - b[ep]
        xe_nm = np.zeros((NPAD, L, D_IN), bfloat16)
        xe_nm[ep, j] = x_bf[es]
        xe_list.append(np.ascontiguousarray(
            xe_nm.transpose(2, 0, 1)))          # [128, NPAD, L]

    xT = np.zeros((NCORES, 128, NPAD), np.float32)
    recipb = np.zeros((NCORES, 128, NPAD), np.float32)
    for c in range(NCORES):
        m = node_of[c] >= 0
        xT[c][:, m] = x[node_of[c][m]].T
        recipb[c][:, m] = recip[node_of[c][m]][None, :]
        recipb[c][:, ~m] = 1.0

    # mask bias: 8 * (ln128 + 0|-60) per (key-partition, b*2+mt)
    m128 = np.zeros((NCORES, 128, NPAD // 128), np.float32)
    for c in range(NCORES):
        for b in range(NSLOTS):
            r = rsizes[c, b]
            for mt in range(2):
                nreal = int(np.clip(r - mt * 128, 0, 128))
                col = 2 * b + mt
                m128[c, :nreal, col] = LN128
                m128[c, nreal:, col] = LN128 + MASK_NEG

    pmask = np.zeros((128, 2), np.float32)
    pmask[0:64, 0] = 1.0 / 16.0
    pmask[64:128, 1] = 1.0 / 16.0

    # ---- weights ----
    g = lambda k: _f32(inputs[k])
    wgt = {}
    wgt["wlT"] = _bf(g("lin_l_w").T)            # [128, 256]
    wgt["wrT"] = _bf(g("lin_r_w").T)
    wgt["bn_g"] = _percol(g("bn_gamma"))
    wgt["bn_b"] = _percol(g("bn_beta"))
    wgt["pmask"] = pmask
    flags = {}
    for l in range(2):
        ipw, ipb = g("in_proj_w")[l], g("in_proj_b")[l]
        Wq, Wk, Wv = ipw[:D], ipw[D:2 * D], ipw[2 * D:]
        bq, bk, bv = ipb[:D], ipb[D:2 * D], ipb[2 * D:]
        Wo, bo = g("out_w")[l], g("out_b")[l]
        bo_f = bo + Wo @ bv
        g1v, b1v = g("ln1_g")[l], g("ln1_b")[l]
        W1g = g("ff1_w")[l] * g1v[None, :]
        b1f = g("ff1_b")[l] + g("ff1_w")[l] @ b1v
        bz = g("ff2_b")[l] + b1v                 # ff2_b + beta1
        wgt[f"wqd{l}"] = _f8(_ktileT(16.0 * Wq))
        wgt[f"wkd{l}"] = _f8(_ktileT(16.0 * Wk))
        wgt[f"wvd{l}"] = _f8(_ktileT(16.0 * Wv))
        wgt[f"wob{l}"] = _bf(_ktileT(Wo))
        wgt[f"w1b{l}"] = _bf(_ktileT(W1g))
        wgt[f"w2b{l}"] = _bf(_ktileT(g("ff2_w")[l]))
        wgt[f"bq16{l}"] = _percol(16.0 * bq)
        wgt[f"bk16{l}"] = _percol(16.0 * bk)
        wgt[f"bo{l}"] = _percol(bo_f)
        wgt[f"b1e{l}"] = _percol(b1f)
        wgt[f"bz{l}"] = _percol(bz)
        wgt[f"g1{l}"] = _percol(g1v)
        wgt[f"g2{l}"] = _percol(g("ln2_g")[l])
        wgt[f"bz2{l}"] = _percol(g("ln2_b")[l])
        flags[f"bo{l}"] = bool(np.abs(bo_f).max() > 0)
        flags[f"bz{l}"] = bool(np.abs(bz).max() > 0)
        flags[f"bz2{l}"] = bool(np.abs(g("ln2_b")[l]).max() > 0)
    clsg = g("cls_w") * g("ln2_g")[1][None, :]
    clsb2 = g("cls_b") + g("cls_w") @ g("ln2_b")[1]
    cd = np.zeros((D, 8), np.float32)
    cd[:, :OUT] = clsg.T
    wgt["clsd"] = _bf(cd.reshape(2, 128, 8).transpose(1, 0, 2))
    wgt["clsb2"] = _f32(clsb2.reshape(OUT, 1))

    meta = dict(SLOT=SLOT, NPAD=NPAD, PT=PT, L=L, flags=flags)

    in_maps = []
    for c in range(NCORES):
        mm = {
            "xe": np.ascontiguousarray(
                xe_list[c].reshape(128, NPAD * L)),
            "xT": _bf(xT[c]),
            "recipb": _f32(recipb[c]),
            "m128": _f32(m128[c]),
        }
        for k, v in wgt.items():
            mm[k] = v
        in_maps.append(mm)

    gather_info = dict(node_of=node_of)
    return meta, in_maps, gather_info


def _build(meta, repeats=1):
    import concourse.bass as _bass
    import concourse.bacc as bacc
    import concourse.bass_isa as bisa
    import concourse.tile as tile
    from concourse import mybir

    NPAD, PT, L = meta["NPAD"], meta["PT"], meta["L"]
    flags = meta["flags"]
    F32, BF16, I16 = mybir.dt.float32, mybir.dt.bfloat16, mybir.dt.int16
    FP8 = mybir.dt.float8e4
    AL = mybir.AluOpType
    AF = mybir.ActivationFunctionType
    PM = mybir.MatmulPerfMode
    RO = bisa.ReduceOp
    NB = NSLOTS  # 4 attention blocks

    DBG = os.environ.get("KERNEL_DEBUG") or None

    nc = bacc.Bacc("TRN2", target_bir_lowering=False, debug=False,
                   enable_asserts=False, num_devices=NCORES)

    def din(name, shape, dt):
        return nc.dram_tensor(name, list(shape), dt, kind="ExternalInput").ap()

    xe_d = din("xe", (128, NPAD * L), BF16)
    xT_d = din("xT", (128, NPAD), BF16)
    recipb_d = din("recipb", (128, NPAD), F32)
    m128_d = din("m128", (128, PT), F32)
    wlT_d = din("wlT", (128, D), BF16)
    wrT_d = din("wrT", (128, D), BF16)
    WD = {}
    for nm in ("bn_g", "bn_b", "pmask"):
        WD[nm] = din(nm, (128, 2), F32)
    for l in range(2):
        for nm in ("wqd", "wkd", "wvd"):
            WD[f"{nm}{l}"] = din(f"{nm}{l}", (128, 2, D), FP8)
        for nm in ("wob", "w1b", "w2b"):
            WD[f"{nm}{l}"] = din(f"{nm}{l}", (128, 2, D), BF16)
        for nm in ("bq16", "bk16", "bo", "b1e", "bz", "g1", "g2", "bz2"):
            WD[f"{nm}{l}"] = din(f"{nm}{l}", (128, 2), F32)
    WD["clsd"] = din("clsd", (128, 2, 8), BF16)
    WD["clsb2"] = din("clsb2", (OUT, 1), F32)

    out_d = nc.dram_tensor("out", [OUT, NPAD], F32, kind="ExternalOutput").ap()
    if DBG:
        dbg_d = nc.dram_tensor("dbg", [128, 2 * NPAD], BF16,
                               kind="ExternalOutput").ap()

    def bc(sl, insert, tail=None):
        """Broadcast AP: insert [0, n] dims after partition dim of 2D slice."""
        ap = [list(sl.ap[0])]
        for n in insert:
            ap.append([0, n])
        for d in (tail if tail is not None else sl.ap[1:]):
            ap.append(list(d))
        return _bass.AP(tensor=sl.tensor, offset=sl.offset, ap=ap)

    with tile.TileContext(nc) as tc:
        import contextlib
        ctx = contextlib.ExitStack()
        consts = ctx.enter_context(tc.tile_pool(name="consts", bufs=1))
        work = ctx.enter_context(tc.tile_pool(name="work", bufs=1))
        psA = ctx.enter_context(tc.tile_pool(name="psA", bufs=1, space="PSUM"))
        psB = ctx.enter_context(tc.tile_pool(name="psB", bufs=2, space="PSUM"))
        dram = ctx.enter_context(tc.tile_pool(name="dram", bufs=1, space="DRAM"))

        def load(name, ap, shape, dt):
            t = consts.tile(list(shape), dt, tag=name)
            nc.sync.dma_start(t[:], ap[:])
            return t

        xe = load("xe", xe_d, (128, NPAD, L), BF16)
        xT = load("xT", xT_d, (128, NPAD), BF16)
        recipb = load("recipb", recipb_d, (128, NPAD), F32)
        m128 = load("m128", m128_d, (128, PT), F32)
        wlT = load("wlT", wlT_d, (128, D), BF16)
        wrT = load("wrT", wrT_d, (128, D), BF16)
        W = {}
        for nm in ("bn_g", "bn_b", "pmask"):
            W[nm] = load(nm, WD[nm], (128, 2), F32)
        for l in range(2):
            for nm in ("wqd", "wkd", "wvd"):
                W[f"{nm}{l}"] = load(f"{nm}{l}", WD[f"{nm}{l}"],
                                     (128, 2, D), FP8)
            for nm in ("wob", "w1b", "w2b"):
                W[f"{nm}{l}"] = load(f"{nm}{l}", WD[f"{nm}{l}"],
                                     (128, 2, D), BF16)
            for nm in ("bq16", "bk16", "bo", "b1e", "bz", "g1", "g2", "bz2"):
                W[f"{nm}{l}"] = load(f"{nm}{l}", WD[f"{nm}{l}"], (128, 2), F32)
        W["clsd"] = load("clsd", WD["clsd"], (128, 2, 8), BF16)
        W["clsb2"] = load("clsb2", WD["clsb2"], (OUT, 1), F32)

        TEN, VEC, SCA, GPS = nc.tensor, nc.vector, nc.scalar, nc.gpsimd

        epsc = consts.tile([128, 1], F32, tag="epsc", name="epsc")
        VEC.memset(epsc[:], EPS)
        eps2c = consts.tile([128, 1], F32, tag="eps2c", name="eps2c")
        VEC.memset(eps2c[:], float(D * D * EPS))

        qmb = work.tile([128, 2, 2, 2, NPAD], FP8, tag="qmb", name="qmb")
        VEC.memset(qmb[:].rearrange("p a b c d -> p (a b c d)"), 0.0)
        vz = work.tile([128, PT, 2, 2, 128], FP8, tag="vz", name="vz")
        VEC.memset(vz[:].rearrange("p a b c d -> p (a b c d)"), 0.0)

        if DBG:
            dbg_done = [False]

            def dbg_dump(name, ap2d):
                if DBG != name or dbg_done[0]:
                    return
                dbg_done[0] = True
                p = ap2d.partition_size()
                f = min(ap2d.free_size(), 2 * NPAD)
                GPS.dma_start(dbg_d[0:p, 0:f], ap2d)
        else:
            def dbg_dump(name, ap2d):
                return

        def ln_block(zw, tag):
            """zw [128, 2, NPAD] bf16 -> (t [128,2,NPAD] bf16, rstdr [128,NPAD] f32)
            with z_norm = t * rstdr."""
            S1 = work.tile([128, NPAD], F32, tag="ln_S1", name="ln_S1")
            mv = work.tile([128, NPAD], F32, tag="ln_mv", name="ln_mv")
            GPS.partition_all_reduce(S1[:], zw[:, 0, :].opt(), 128, RO.add)
            GPS.partition_all_reduce(mv[:], zw[:, 1, :].opt(), 128, RO.add)
            VEC.tensor_tensor(S1[:], S1[:], mv[:], AL.add)
            sqd = work.tile([128, 2, NPAD], BF16, tag="ln_sqd", name="ln_sqd")
            VEC.tensor_tensor(sqd[:], zw[:], zw[:], AL.mult)
            S2 = work.tile([128, NPAD], F32, tag="ln_S2", name="ln_S2")
            GPS.partition_all_reduce(S2[:], sqd[:, 0, :].opt(), 128, RO.add)
            GPS.partition_all_reduce(mv[:], sqd[:, 1, :].opt(), 128, RO.add)
            VEC.tensor_tensor(S2[:], S2[:], mv[:], AL.add)
            VEC.tensor_tensor(mv[:], S1[:], S1[:], AL.mult)
            VEC.scalar_tensor_tensor(mv[:], S2[:], float(D), mv[:],
                                     AL.mult, AL.subtract)
            srt = work.tile([128, NPAD], F32, tag="ln_srt", name="ln_srt")
            SCA.activation(srt[:], mv[:], AF.Sqrt,
                           bias=eps2c[:, 0:1], scale=1.0)
            rstdr = work.tile([128, NPAD], F32, tag=f"rstdr{tag}",
                              name=f"rstdr{tag}")
            VEC.reciprocal(rstdr[:], srt[:])
            t = work.tile([128, 2, NPAD], BF16, tag=f"lnt{tag}",
                          name=f"lnt{tag}")
            VEC.scalar_tensor_tensor(t[:], zw[:], float(D),
                                     bc(S1[:], [2]), AL.mult, AL.subtract)
            return t, rstdr

        for rep in range(repeats):
            # ============ SAGE ============
            aggT = work.tile([128, NPAD], F32, tag="ln_S2", name="aggT")
            VEC.tensor_reduce(aggT[:], xe[:], mybir.AxisListType.X, AL.add)
            aggTb = work.tile([128, NPAD], BF16, tag="uw", name="aggTb")
            VEC.tensor_tensor(aggTb[:], aggT[:], recipb[:], AL.mult)
            dbg_dump("agg", aggTb[:])

            # ============ h + BN ============
            h_ps = psA.tile([128, 2, NPAD], F32, tag="psA", name="psA")
            for kt in range(2):
                for ch in range(2):
                    sl = slice(ch * 512, (ch + 1) * 512)
                    TEN.matmul(h_ps[:, kt, sl],
                               wlT[:, kt * 128:(kt + 1) * 128],
                               aggTb[:, sl], start=True, stop=False)
                    TEN.matmul(h_ps[:, kt, sl],
                               wrT[:, kt * 128:(kt + 1) * 128],
                               xT[:, sl], start=False, stop=True)
            hT = work.tile([128, 2, NPAD], BF16, tag="lnt2", name="hT")
            VEC.tensor_copy(hT[:].rearrange("p a b -> p (a b)"),
                            h_ps[:].rearrange("p a b -> p (a b)"))
            ssum = work.tile([128, 4], F32, tag="ssum", name="ssum")
            VEC.tensor_reduce(ssum[:, 0:2], hT[:], mybir.AxisListType.X,
                              AL.add)
            sqd = work.tile([128, 2, NPAD], BF16, tag="ln_sqd", name="bn_sqd")
            VEC.tensor_tensor(sqd[:], hT[:], hT[:], AL.mult)
            VEC.tensor_reduce(ssum[:, 2:4], sqd[:], mybir.AxisListType.X,
                              AL.add)

            bn_in = dram.tile([128, 4], F32, tag="bn_in", name="bn_in")
            bn_out = dram.tile([128, 4], F32, tag="bn_out", name="bn_out")
            GPS.dma_start(bn_in[:], ssum[:])
            GPS.collective_compute(
                "AllReduce", AL.add,
                replica_groups=[list(range(NCORES))],
                ins=[bn_in[:].opt()], outs=[bn_out[:].opt()])
            ssg = work.tile([128, 4], F32, tag="ssg", name="ssg")
            GPS.dma_start(ssg[:], bn_out[:])

            cf = work.tile([128, 8], F32, tag="bncf", name="bncf")
            mean, e2 = cf[:, 0:2], cf[:, 2:4]
            a_bn, b_bn = cf[:, 4:6], cf[:, 6:8]
            VEC.tensor_scalar_mul(mean, ssg[:, 0:2], 1.0 / N)
            VEC.tensor_scalar_mul(e2, ssg[:, 2:4], 1.0 / N)
            var = work.tile([128, 2], F32, tag="bnvar", name="bnvar")
            VEC.tensor_tensor(var[:], mean, mean, AL.mult)
            VEC.tensor_tensor(var[:], e2, var[:], AL.subtract)
            SCA.activation(var[:], var[:], AF.Sqrt, bias=epsc[:, 0:1], scale=1.0)
            rbn = work.tile([128, 2], F32, tag="bnr", name="bnr")
            VEC.reciprocal(rbn[:], var[:])
            VEC.tensor_tensor(a_bn, W["bn_g"][:], rbn[:], AL.mult)
            VEC.tensor_tensor(b_bn, mean, a_bn, AL.mult)
            VEC.tensor_tensor(b_bn, W["bn_b"][:], b_bn, AL.subtract)

            h1 = work.tile([128, 2, NPAD], BF16, tag="inT", name="h1")
            for kt in range(2):
                VEC.tensor_scalar(h1[:, kt, :].opt(), hT[:, kt, :].opt(),
                                  a_bn[:, kt:kt + 1], b_bn[:, kt:kt + 1],
                                  AL.mult, AL.add)
            VEC.tensor_scalar_max(
                h1[:].rearrange("p a b -> p (a b)"),
                h1[:].rearrange("p a b -> p (a b)"), 0.0)
            h1f = work.tile([128, 2, NPAD], FP8, tag="inF", name="h1f")
            VEC.tensor_copy(h1f[:].rearrange("p a b -> p (a b)"),
                            h1[:].rearrange("p a b -> p (a b)"))
            dbg_dump("h1", h1[:].rearrange("p a b -> p (a b)"))

            inT, inF = h1, h1f
            # ============ layers ============
            for l in range(2):
                # ---- q ----
                q_ps = psA.tile([128, 2, NPAD], F32, tag="psA", name="psA")
                for kt in range(2):
                    for ch in range(2):
                        sl = slice(ch * 512, (ch + 1) * 512)
                        TEN.matmul(q_ps[:, kt, sl],
                                   W[f"wqd{l}"][:, :, kt * 128:(kt + 1) * 128],
                                   inF[:, :, sl], start=True, stop=True,
                                   perf_mode=PM.DoubleRow)
                dbg_dump("q", q_ps[:].rearrange("p a b -> p (a b)"))
                for hp in range(2):
                    for par in range(2):
                        VEC.scalar_tensor_tensor(
                            qmb[:, hp, par, hp, :].opt(),
                            q_ps[:, hp, :].opt(),
                            W[f"bq16{l}"][:, hp:hp + 1],
                            bc(W["pmask"][:, par:par + 1], [], [[0, NPAD]]),
                            AL.add, AL.mult)
                # ---- k ----
                k_ps = psA.tile([128, 2, NPAD], F32, tag="psA", name="psA")
                for kt in range(2):
                    for ch in range(2):
                        sl = slice(ch * 512, (ch + 1) * 512)
                        TEN.matmul(k_ps[:, kt, sl],
                                   W[f"wkd{l}"][:, :, kt * 128:(kt + 1) * 128],
                                   inF[:, :, sl], start=True, stop=True,
                                   perf_mode=PM.DoubleRow)
                kT = work.tile([128, 2, NPAD], FP8, tag="kT", name="kT")
                for kt in range(2):
                    VEC.tensor_scalar(kT[:, kt, :].opt(), k_ps[:, kt, :].opt(),
                                      W[f"bk16{l}"][:, kt:kt + 1], 1.0 / 16.0,
                                      AL.add, AL.mult)
                dbg_dump("kT", kT[:].rearrange("p a b -> p (a b)"))
                # ---- V (plain bf16, zero-padded per-(kt,par) blocks) ----
                for pt in range(PT):
                    v_ps = psB.tile([128, D], F32, tag="sm", name="psV")
                    TEN.matmul(v_ps[:], inF[:, :, pt * 128:(pt + 1) * 128],
                               W[f"wvd{l}"][:], start=True, stop=True,
                               perf_mode=PM.DoubleRow)
                    vv = v_ps[:].rearrange("p (a b c) -> p a b c", a=2, b=2)
                    for par in range(2):
                        VEC.tensor_scalar_mul(
                            vz[:, pt, :, par, 64 * par:64 * par + 64].opt(),
                            vv[:, :, par, :].opt(), 1.0 / 16.0)
                # ---- scores + exp ----
                expw = work.tile([128, NB, 2, 2, 2, 256], BF16, tag="expw",
                                 name="expw")
                for b in range(NB):
                    for mt in range(2):
                        sc_ps = psB.tile([128, 2, 2, 256], F32, tag="sm",
                                         name="psS")
                        kc = (2 * b + mt) * 128
                        for par in range(2):
                            TEN.matmul(sc_ps[:, par, :, :],
                                       kT[:, :, kc:kc + 128],
                                       qmb[:, :, par, :,
                                           b * 256:(b + 1) * 256],
                                       start=True, stop=True,
                                       perf_mode=PM.DoubleRow)
                        col = 2 * b + mt
                        SCA.activation(
                            expw[:, b, mt, :, :, :].rearrange(
                                "p a b c -> p (a b c)"),
                            sc_ps[:].rearrange("p a b c -> p (a b c)"),
                            AF.Exp, bias=m128[:, col:col + 1], scale=0.125)
                dbg_dump("exp", expw[:, 0, :, :, :, :].rearrange("p a b c d -> p (a b c d)"))
                # ---- denom -> P~ (x128, fp8) ----
                den = work.tile([128, NB, 2, 2, 2, 256], BF16, tag="den",
                                name="den")
                GPS.partition_all_reduce(
                    den[:].rearrange("p a b c d e -> p (a b c d e)"),
                    expw[:].rearrange("p a b c d e -> p (a b c d e)"),
                    128, RO.add)
                den2 = work.tile([128, NB, 2, 2, 256], F32, tag="den2",
                                 name="den2")
                VEC.tensor_tensor(den2[:].opt(), den[:, :, 0, :, :, :].opt(),
                                  den[:, :, 1, :, :, :].opt(), AL.add)
                VEC.reciprocal(den2[:].rearrange("p a b c d -> p (a b c d)"),
                               den2[:].rearrange("p a b c d -> p (a b c d)"))
                ptil = work.tile([128, NB, 2, 2, 2, 256], FP8, tag="ptil",
                                 name="ptil")
                for b in range(NB):
                    d2b = den2[:, b, :, :, :].opt()
                    VEC.scalar_tensor_tensor(
                        ptil[:, b, :, :, :, :].opt(),
                        expw[:, b, :, :, :, :].opt(), 128.0,
                        bc(d2b, [2]), AL.mult, AL.mult)
                dbg_dump("ptil", expw[:, 0, :, :, :, :].rearrange("p a b c d -> p (a b c d)"))
                # ---- ovT ----
                voT = work.tile([128, 2, NPAD], BF16, tag="ln_sqd",
                                 name="voT")
                for b in range(NB):
                    ops = psB.tile([128, 2, 256], F32, tag="sm", name="psO")
                    for kt in range(2):
                        for par in range(2):
                            TEN.matmul(
                                ops[:, kt, :],
                                vz[:, 2 * b:2 * b + 2, kt, par, :],
                                ptil[:, b, :, par, kt, :],
                                start=(par == 0), stop=(par == 1),
                                perf_mode=PM.DoubleRow)
                    VEC.tensor_copy(voT[:, :, b * 256:(b + 1) * 256], ops[:])
                dbg_dump("voT", voT[:].rearrange("p a b -> p (a b)"))
                # ---- oproj + residual -> z1 ----
                o_ps = psA.tile([128, 2, NPAD], F32, tag="psA", name="psA")
                for kt in range(2):
                    for ch in range(2):
                        sl = slice(ch * 512, (ch + 1) * 512)
                        for kk in range(2):
                            TEN.matmul(
                                o_ps[:, kt, sl],
                                W[f"wob{l}"][:, kk,
                                             kt * 128:(kt + 1) * 128],
                                voT[:, kk, sl].opt(),
                                start=(kk == 0), stop=(kk == 1))
                z1 = work.tile([128, 2, NPAD], BF16, tag="z1", name="z1")
                for kt in range(2):
                    VEC.scalar_tensor_tensor(z1[:, kt, :].opt(),
                                             o_ps[:, kt, :].opt(),
                                             1.0 / 128.0,
                                             inT[:, kt, :].opt(),
                                             AL.mult, AL.add)
                if flags[f"bo{l}"]:
                    for kt in range(2):
                        VEC.tensor_scalar(z1[:, kt, :].opt(), z1[:, kt, :].opt(),
                                          W[f"bo{l}"][:, kt:kt + 1], None,
                                          AL.add)
                dbg_dump("z1", z1[:].rearrange("p a b -> p (a b)"))
                # ---- LN1 -> z1n ----
                t1, rstdr1 = ln_block(z1, "1")
                z1n = work.tile([128, 2, NPAD], BF16, tag="z1n", name="z1n")
                VEC.tensor_tensor(z1n[:], t1[:], bc(rstdr1[:], [2]), AL.mult)
                dbg_dump("z1n", z1n[:].rearrange("p a b -> p (a b)"))
                # ---- FFN1 ----
                f_ps = psA.tile([128, 2, NPAD], F32, tag="psA", name="psA")
                for kt in range(2):
                    for ch in range(2):
                        sl = slice(ch * 512, (ch + 1) * 512)
                        for kk in range(2):
                            TEN.matmul(
                                f_ps[:, kt, sl],
                                W[f"w1b{l}"][:, kk,
                                             kt * 128:(kt + 1) * 128],
                                z1n[:, kk, sl].opt(),
                                start=(kk == 0), stop=(kk == 1))
                relu1 = work.tile([128, 2, NPAD], BF16, tag="z1n",
                                  name="relu1")
                for kt in range(2):
                    VEC.tensor_scalar(relu1[:, kt, :].opt(),
                                      f_ps[:, kt, :].opt(),
                                      W[f"b1e{l}"][:, kt:kt + 1], 0.0,
                                      AL.add, AL.max)
                # ---- FFN2 + z2 ----
                f2_ps = psA.tile([128, 2, NPAD], F32, tag="psA", name="psA")
                for kt in range(2):
                    for ch in range(2):
                        sl = slice(ch * 512, (ch + 1) * 512)
                        for kk in range(2):
                            TEN.matmul(
                                f2_ps[:, kt, sl],
                                W[f"w2b{l}"][:, kk,
                                             kt * 128:(kt + 1) * 128],
                                relu1[:, kk, sl].opt(),
                                start=(kk == 0), stop=(kk == 1))
                z2 = work.tile([128, 2, NPAD], BF16, tag="z1", name="z2")
                uw = work.tile([128, NPAD], BF16, tag="uw", name="uw")
                for kt in range(2):
                    VEC.scalar_tensor_tensor(uw[:], t1[:, kt, :].opt(),
                                             W[f"g1{l}"][:, kt:kt + 1],
                                             rstdr1[:], AL.mult, AL.mult)
                    VEC.scalar_tensor_tensor(z2[:, kt, :].opt(),
                                             f2_ps[:, kt, :].opt(),
                                             1.0, uw[:],
                                             AL.mult, AL.add)
                if flags[f"bz{l}"]:
                    for kt in range(2):
                        VEC.tensor_scalar(z2[:, kt, :].opt(), z2[:, kt, :].opt(),
                                          W[f"bz{l}"][:, kt:kt + 1], None,
                                          AL.add)
                dbg_dump("z2", z2[:].rearrange("p a b -> p (a b)"))
                # ---- LN2 ----
                t2, rstdr2 = ln_block(z2, "2")
                if l == 0:
                    h2 = work.tile([128, 2, NPAD], BF16, tag="inT",
                                   name="h2")
                    for kt in range(2):
                        VEC.scalar_tensor_tensor(h2[:, kt, :].opt(),
                                                 t2[:, kt, :].opt(),
                                                 W["g20"][:, kt:kt + 1],
                                                 rstdr2[:], AL.mult, AL.mult)
                    if flags["bz20"]:
                        for kt in range(2):
                            VEC.tensor_scalar(h2[:, kt, :].opt(), h2[:, kt, :].opt(),
                                              W["bz20"][:, kt:kt + 1], None,
                                              AL.add)
                    h2f = work.tile([128, 2, NPAD], FP8, tag="inF",
                                    name="h2f")
                    VEC.tensor_copy(h2f[:].rearrange("p a b -> p (a b)"),
                                    h2[:].rearrange("p a b -> p (a b)"))
                    dbg_dump("h2", h2[:].rearrange("p a b -> p (a b)"))
                    inT, inF = h2, h2f
                else:
                    z2n = work.tile([128, 2, NPAD], BF16, tag="z1n",
                                    name="z2n")
                    VEC.tensor_tensor(z2n[:], t2[:], bc(rstdr2[:], [2]),
                                      AL.mult)
                    c_ps = psB.tile([8, NPAD], F32, tag="sm", name="psC")
                    for ch in range(2):
                        sl = slice(ch * 512, (ch + 1) * 512)
                        for kk in range(2):
                            TEN.matmul(c_ps[:, sl], W["clsd"][:, kk, :],
                                       z2n[:, kk, sl].opt(),
                                       start=(kk == 0), stop=(kk == 1))
                    outb = work.tile([OUT, NPAD], F32, tag="ln_S1",
                                     name="outb")
                    VEC.tensor_scalar(outb[:], c_ps[0:OUT, :], 1.0,
                                      W["clsb2"][:, 0:1], AL.mult, AL.add)
                    nc.sync.dma_start(out_d[:], outb[:])
        ctx.close()

    nc.finalize()
    return nc


def _run(meta, in_maps, repeats=1):
    from concourse import bass_utils
    key = (meta["NPAD"], meta["L"], repeats,
           os.environ.get("KERNEL_DEBUG") or None)
    if key not in _cache:
        _cache[key] = _build(meta, repeats=repeats)
    nc = _cache[key]
    res = bass_utils.run_bass_kernel_spmd(
        nc, in_maps, core_ids=list(range(NCORES)), trace=False)
    return res


def kernel(**inputs):
    meta, in_maps, gi = _prep(inputs)
    repeats = int(os.environ.get("KERNEL_REPEATS", "1"))
    res = _run(meta, in_maps, repeats=repeats)
    node_of = gi["node_of"]
    out = np.zeros((N, OUT), np.float32)
    for c in range(NCORES):
        m = node_of[c] >= 0
        out[node_of[c][m]] = np.ascontiguousarray(res.results[c]["out"].T)[m]
    return out
